# revision 1
# baseline (speedup 1.0000x reference)
"""Trainium2 Bass kernel for the DTFA (dual-attention SE + threshold
decomposition) module.

Math (per batch b):
  zt = SE(mean_T(x))            # [C, F]
  zf = SE(mean_F(x))            # [C, T]
  out1[t,f] = sum_c wf[c]*zf[c,t]*zt[c,f] + bf          (rank-C matmul)
  dcomp[k]  = where(out1 > thr_k, out1, 0), k=1..23
  out[c]    = (sum_k wf2[c,k]*dcomp[k] + bf2[c]) * x[c]

Sharding: pure data-parallel, 2 batches per core on 8 cores.

Pass 1 streams the input as [128t, 8c*256f] tiles (1 MB DMAs): T-sums via
PE ones-matmuls into [1, 2048] PSUM rows, F-sums via DVE tensor_reduce.
Pass 2 processes 1024-pixel block pairs (blocks 2i, 2i+1): a PE
broadcast-matmul replicates out1 into 2x(ones + 23 thresholds) x 2
batches rows ([112, 512] PSUM, bias folded via the ones row), one fused
DVE scalar_tensor_tensor forms (x > thr) * x, a block-diagonal [bf2|wf2]
matmul produces out2 for both batches at once, and a single DVE
tensor_tensor multiplies by the re-streamed input.
"""

import numpy as np

B, C, OC, T, F = 16, 64, 16, 256, 256
N_THR = 23
N_CORES = 8
BL = B // N_CORES  # local batches per core = 2
PIX = T * F        # 65536 per batch
NPAIR = 64         # pairs of adjacent 512-pix blocks (t-quads) per batch

_CACHE = {}


def _host_constants(w1, b1, w2, b2, wf, bf, wf2, bf2):
    f32 = np.float32
    c = {}
    # SE weights. lhsT layout [K, M]; fold the 1/256 mean scale into w1.
    c["w1Ts"] = np.ascontiguousarray(w1.T.astype(f32) / 256.0)          # [64, 16]
    c["w2T"] = np.ascontiguousarray(w2.T.astype(f32))                   # [16, 64]
    c["b1c"] = b1.astype(f32).reshape(OC, 1)
    c["b2c"] = b2.astype(f32).reshape(C, 1)
    c["wfcol"] = wf.astype(f32).reshape(C, 1)                            # [64, 1]
    bf_s = float(np.asarray(bf).reshape(-1)[0])

    # Broadcast matmul weights: xB[m, n] = sum_k bcastW[k, m] * xflat[k, n]
    # xflat rows: 0 = A even-block pix, 1 = B even, 2 = A odd, 3 = B odd,
    #             4 = ones.
    # xB rows m: 0-47 -> even block (g=0), 64-111 -> odd block (g=1);
    # within a 48-group: r = b*24 + k.  k=0 is the bias/ones row.
    bcastW5 = np.zeros((5, 112), f32)
    for m in range(112):
        if 48 <= m < 64:
            continue
        g, r = (0, m) if m < 48 else (1, m - 64)
        b_loc, k = divmod(r, 24)
        if k == 0:
            bcastW5[4, m] = 1.0
        else:
            bcastW5[2 * g + b_loc, m] = 1.0
            bcastW5[4, m] = bf_s
    c["bcastW"] = bcastW5

    # Threshold column for the fused (x > thr) * x op.
    thrcol = np.zeros((112, 1), f32)
    for m in range(112):
        if 48 <= m < 64:
            thrcol[m, 0] = 1e30
            continue
        r = m if m < 48 else m - 64
        k = r % 24
        thrcol[m, 0] = -1e30 if k == 0 else f32(k * (k + 1) / 600.0)
    c["thrcol"] = thrcol

    # Block-diagonal [bf2 | wf2] weights for the decomposition matmul.
    # rows (b, k) at bases 0 and 64; cols m = b*64 + c.
    wbd = np.zeros((112, 128), f32)
    for base in (0, 64):
        for b_loc in range(2):
            for k in range(24):
                row = base + 24 * b_loc + k
                cols = slice(64 * b_loc, 64 * b_loc + 64)
                wbd[row, cols] = bf2.astype(f32) if k == 0 else wf2[:, k - 1].astype(f32)
    # Pack small constants into two [128, N] arrays (one SBUF tile each).
    packA = np.zeros((128, 96), f32)
    packA[0:64, 0:16] = c.pop("w1Ts")
    packA[0:16, 16:80] = c.pop("w2T")
    packA[0:16, 80:81] = c.pop("b1c")
    packA[0:64, 81:82] = c.pop("b2c")
    packA[0:64, 82:83] = c.pop("wfcol")
    packA[:, 83:84] = 1.0                      # ones128
    packA[0:112, 84:85] = thrcol
    c.pop("thrcol")
    packB = np.zeros((128, 368), f32)
    packB[0:112, 0:128] = wbd
    packB[:, 128:256] = np.eye(128, dtype=f32)
    packB[0:5, 256:368] = c.pop("bcastW")
    c["packA"] = packA
    c["packB"] = packB
    c["ones8k"] = np.ones((1, 8192), f32)
    return c


CONST_SHAPES = {
    "packA": (128, 96), "packB": (128, 368), "ones8k": (1, 8192),
}


def _build_nc(reps=1, phase="all"):
    from contextlib import ExitStack, nullcontext

    import concourse.bass as bass
    import concourse.bacc as bacc
    import concourse.tile as tile
    from concourse import mybir

    f32 = mybir.dt.float32
    Alu = mybir.AluOpType
    Act = mybir.ActivationFunctionType

    nc = bacc.Bacc("TRN2", target_bir_lowering=False, debug=False)
    feat = nc.dram_tensor("feat", [BL, C, T, F], f32, kind="ExternalInput")
    outp = nc.dram_tensor("outp", [BL, C, T, F], f32, kind="ExternalOutput")
    cts = {
        name: nc.dram_tensor(name, list(shape), f32, kind="ExternalInput")
        for name, shape in CONST_SHAPES.items()
    }

    with tile.TileContext(nc) as tc, ExitStack() as ctx:
        cpool = ctx.enter_context(tc.tile_pool(name="consts", bufs=1))
        cA = cpool.tile([128, 96], f32, tag="packA", name="c_packA")
        nc.gpsimd.dma_start(out=cA[:], in_=cts["packA"][:])
        cB = cpool.tile([128, 368], f32, tag="packB", name="c_packB")
        nc.gpsimd.dma_start(out=cB[:], in_=cts["packB"][:])
        sb = {
            "w1Ts": cA[0:64, 0:16], "w2T": cA[0:16, 16:80],
            "b1c": cA[0:16, 80:81], "b2c": cA[0:64, 81:82],
            "wfcol": cA[0:64, 82:83], "ones128": cA[:, 83:84],
            "thrcol": cA[0:112, 84:85],
            "wbd": cB[0:112, 0:128], "ident128": cB[:, 128:256],
            "bcastW": cB[0:5, 256:368],
        }

        loop_cm = tc.For_i(0, reps, 1) if reps > 1 else nullcontext()
        ctx.enter_context(loop_cm)
        persist = ctx.enter_context(tc.tile_pool(name="persist", bufs=1))
        p1pool = ctx.enter_context(tc.tile_pool(name="p1feat", bufs=3))

        small64 = persist.tile([64, 4096], f32, tag="small64", name="small64")
        def w64(idx, rows=64):
            return small64[0:rows, 256 * idx : 256 * idx + 256]
        ztsum = [w64(0), w64(1)]
        zfsum = [w64(2), w64(3)]
        zfpart_all = persist.tile([128, 256], f32, tag="zfpart", name="zfpart")
        zfpart = [
            [zfpart_all[:, 64 * (2 * b + h) : 64 * (2 * b + h) + 64]
             for h in range(2)]
            for b in range(BL)
        ]
        x128 = persist.tile([128, 1024], f32, tag="x128", name="x128")
        if phase == "p2":
            nc.gpsimd.memset(x128[:], 0.0)
        x_sb = [
            [x128[:, 256 * (2 * b + m) : 256 * (2 * b + m) + 256]
             for m in range(2)]
            for b in range(BL)
        ]

        # ---------------- Pass 1: row/col sums ----------------
        # 2 MB input tiles (16 channels); T-sums accumulate in [1, 2048]
        # PSUM rows (8 channels each), then ACT-copy to an SBUF row and
        # DMA-redistribute to [8, 256].
        with tc.tile_pool(name="ps_tsum", bufs=2, space="PSUM") as ppt:
            for b in range(BL if phase != "p2" else 0):
                for q16 in range(C // 16):  # 16-channel groups
                    fts = []
                    for h in range(2):
                        ft = p1pool.tile([128, 16, F], f32, tag="ft", name="ft")
                        src = feat[b, 16 * q16 : 16 * q16 + 16,
                                   128 * h : 128 * h + 128, :]
                        eng = nc.sync if (q16 + h) % 2 == 0 else nc.scalar
                        eng.dma_start(out=ft[:], in_=src.transpose([1, 0, 2]))
                        nc.vector.tensor_reduce(
                            out=zfpart[b][h][:, 16 * q16 : 16 * q16 + 16],
                            in_=ft[:],
                            axis=mybir.AxisListType.X,
                            op=Alu.add,
                        )
                        fts.append(ft)
                    for half in range(2):  # two channel-octets
                        jj = 2 * q16 + half
                        tsum = ppt.tile([1, 2048], f32, tag="tsum", name="tsum")
                        for s in range(4):
                            for h in range(2):
                                nc.tensor.matmul(
                                    tsum[:, 512 * s : 512 * s + 512],
                                    sb["ones128"],
                                    fts[h][:, 8 * half + 2 * s : 8 * half + 2 * s + 2, :],
                                    start=(h == 0), stop=(h == 1),
                                )
                        ztrow = persist.tile([1, 2048], f32, tag="ztrow",
                                             name="ztrow", bufs=2)
                        nc.scalar.copy(ztrow[:], tsum[:])
                        nc.gpsimd.dma_start(
                            out=ztsum[b][8 * jj : 8 * jj + 8, :], in_=ztrow[:]
                        )

        with tc.tile_pool(name="ps_tp", bufs=2, space="PSUM") as pptp:
            for b in range(BL if phase != "p2" else 0):
                for h in range(2):
                    tp = pptp.tile([C, 128], f32, tag="tp")
                    nc.tensor.transpose(tp[:], zfpart[b][h], sb["ident128"])
                    nc.scalar.copy(zfsum[b][:, 128 * h : 128 * h + 128], tp[:])

        # ---------------- SE branches + out1 ----------------
        def se_branch(zin, sidx):
            h1p = ppse.tile([OC, 256], f32, tag="h1p")
            nc.tensor.matmul(h1p[:], sb["w1Ts"], zin)
            h1s = small64[0:OC, 256 * (10 + sidx) : 256 * (10 + sidx) + 256]
            nc.scalar.activation(h1s, h1p[:], Act.Relu,
                                 bias=sb["b1c"], scale=1.0)
            h2p = ppse.tile([C, 256], f32, tag="h2p")
            nc.tensor.matmul(h2p[:], sb["w2T"], h1s)
            zout = w64(4 + sidx)
            nc.scalar.activation(zout, h2p[:], Act.Sigmoid,
                                 bias=sb["b2c"], scale=1.0)
            return zout

        with tc.tile_pool(name="ps_se", bufs=1, space="PSUM") as ppse:
            for b in range(BL if phase != "p2" else 0):
                zt = se_branch(ztsum[b], 2 * b)
                zf = se_branch(zfsum[b], 2 * b + 1)
                wfzf = w64(8 + b)
                nc.vector.tensor_scalar_mul(wfzf, zf, sb["wfcol"])
                for m in range(2):
                    o1 = ppse.tile([128, F], f32, tag="o1")
                    nc.tensor.matmul(
                        o1[:], wfzf[:, 128 * m : 128 * m + 128], zt
                    )
                    nc.scalar.copy(x_sb[b][m], o1[:])

        # ---------------- x_flat: [5, 8192] per quarter ----------------
        # Quarter q covers pairs 16q..16q+15 (t-rows 64q..64q+63).  Row
        # layout: 0 = A even blocks, 1 = B even, 2 = A odd, 3 = B odd,
        # 4 = ones.  Even block of pair p = t-rows {4p, 4p+1}; odd =
        # {4p+2, 4p+3}.
        if phase == "p1":
            xfpool = None
        else:
            xfpool = ctx.enter_context(tc.tile_pool(name="xflat", bufs=2))
        xflat = []
        for q in range(8 if phase != "p1" else 0):  # groups of 8 pairs
            xf = xfpool.tile([5, 4096], f32, tag="xf", name=f"xf{q}")
            m, tbase = divmod(q, 4)  # x_sb half-tile and 32-row base
            for par, (b_loc, off) in enumerate(
                [(0, 0), (1, 0), (0, 2), (1, 2)]
            ):
                srct = x_sb[b_loc][m]
                pitch = srct.ap[0][0]
                for sub in range(2):
                    row0 = 32 * tbase + off + sub
                    s0 = srct[row0 : row0 + 1, :]
                    src_ap = bass.AP(
                        tensor=s0.tensor, offset=s0.offset,
                        ap=[[4 * pitch, 8], [1, 256]],
                    )
                    d0 = xf[par : par + 1, :]
                    dst_ap = bass.AP(
                        tensor=d0.tensor, offset=d0.offset + 256 * sub,
                        ap=[[4096, 1], [512, 8], [1, 256]],
                    )
                    nc.gpsimd.dma_start(out=dst_ap, in_=src_ap)
            nc.gpsimd.dma_start(out=xf[4:5, :], in_=cts["ones8k"][0:1, 0:4096])
            xflat.append(xf)

        # ---------------- Pass 2 ----------------
        p2pool = ctx.enter_context(tc.tile_pool(name="p2feat", bufs=3))
        opool = ctx.enter_context(tc.tile_pool(name="outs", bufs=3))
        xbspool = ctx.enter_context(tc.tile_pool(name="xbs", bufs=2))
        dcpool = ctx.enter_context(tc.tile_pool(name="dcomp", bufs=2))
        ppxb = ctx.enter_context(tc.tile_pool(name="ps_xb", bufs=2, space="PSUM"))
        ppg = ctx.enter_context(tc.tile_pool(name="ps_g", bufs=2, space="PSUM"))

        # Two pairs (8 t-rows, 1 MB) per input/output DMA.
        for grp in range(NPAIR // 2 if phase != "p1" else 0):
            ft2 = p2pool.tile([128, 8, F], f32, tag="ft2", name="ft2")
            nc.sync.dma_start(out=ft2[:], in_=feat[:, :, 8 * grp : 8 * grp + 8, :])
            ot = opool.tile([128, 8, F], f32, tag="ot", name="ot")
            for ii in range(2):
                i = 2 * grp + ii
                q, r = divmod(i, 8)
                xB = ppxb.tile([112, 512], f32, tag="xB")
                nc.tensor.matmul(
                    xB[:], sb["bcastW"], xflat[q][:, 512 * r : 512 * r + 512]
                )
                xBs = xbspool.tile([112, 512], f32, tag="xBs")
                nc.scalar.copy(xBs[:], xB[:])
                dc = dcpool.tile([112, 512], f32, tag="dc")
                nc.vector.scalar_tensor_tensor(
                    out=dc[:], in0=xBs[:], scalar=sb["thrcol"], in1=xB[:],
                    op0=Alu.is_gt, op1=Alu.mult,
                )
                gp = ppg.tile([128, 1024], f32, tag="gp")
                for g in (0, 1):
                    nc.tensor.matmul(
                        gp[:, 512 * g : 512 * g + 512],
                        sb["wbd"][64 * g : 64 * g + 48, :],
                        dc[64 * g : 64 * g + 48, :],
                    )
                nc.vector.tensor_tensor(
                    out=ot[:, 4 * ii : 4 * ii + 4, :],
                    in0=gp[:].rearrange("p (a b) -> p a b", a=4),
                    in1=ft2[:, 4 * ii : 4 * ii + 4, :], op=Alu.mult,
                )
            nc.scalar.dma_start(
                out=outp[:, :, 8 * grp : 8 * grp + 8, :], in_=ot[:]
            )

    nc.finalize()
    return nc


def _get_nc(reps=1, phase="all"):
    key = ("nc", reps, phase)
    if key not in _CACHE:
        _CACHE[key] = _build_nc(reps, phase)
    return _CACHE[key]


def _make_runner(nc, n_cores):
    """Cached jitted shard_map executor for `nc` (mirrors
    bass2jax.run_bass_via_pjrt but reusable across calls)."""
    import jax
    from jax.sharding import Mesh, PartitionSpec
    from jax.experimental.shard_map import shard_map
    from concourse import bass2jax, mybir

    bass2jax.install_neuronx_cc_hook()

    partition_name = (
        nc.partition_id_tensor.name if nc.partition_id_tensor else None
    )
    in_names, out_names, out_avals, zero_outs = [], [], [], []
    for alloc in nc.m.functions[0].allocations:
        if not isinstance(alloc, mybir.MemoryLocationSet):
            continue
        name = alloc.memorylocations[0].name
        if alloc.kind == "ExternalInput":
            if name != partition_name:
                in_names.append(name)
        elif alloc.kind == "ExternalOutput":
            out_names.append(name)
            shape = tuple(alloc.tensor_shape)
            dtype = mybir.dt.np(alloc.dtype)
            out_avals.append(jax.core.ShapedArray(shape, dtype))
            zero_outs.append(np.zeros(shape, dtype))
    n_params = len(in_names)
    all_in_names = in_names + out_names
    if partition_name is not None:
        all_in_names = all_in_names + [partition_name]
    donate = tuple(range(n_params, n_params + len(out_names)))

    def _body(*args):
        operands = list(args)
        if partition_name is not None:
            operands.append(bass2jax.partition_id_tensor())
        outs = bass2jax._bass_exec_p.bind(
            *operands,
            out_avals=tuple(out_avals),
            in_names=tuple(all_in_names),
            out_names=tuple(out_names),
            lowering_input_output_aliases=(),
            sim_require_finite=True,
            sim_require_nnan=True,
            nc=nc,
        )
        return tuple(outs)

    devices = jax.devices()[:n_cores]
    mesh = Mesh(np.asarray(devices), ("core",))
    specs = (PartitionSpec("core"),) * (n_params + len(out_names))
    sharded = jax.jit(
        shard_map(_body, mesh=mesh, in_specs=specs,
                  out_specs=(PartitionSpec("core"),) * len(out_names),
                  check_rep=False),
        donate_argnums=donate, keep_unused=True,
    )

    def run(in_maps):
        per_core = [[np.asarray(m[name]) for name in in_names] for m in in_maps]
        concat_in = [
            np.concatenate([per_core[c][i] for c in range(n_cores)], axis=0)
            for i in range(n_params)
        ]
        concat_zeros = [
            np.zeros((n_cores * z.shape[0], *z.shape[1:]), z.dtype)
            for z in zero_outs
        ]
        out_arrs = sharded(*concat_in, *concat_zeros)
        return [
            {
                name: np.asarray(out_arrs[i]).reshape(n_cores, *out_avals[i].shape)[c]
                for i, name in enumerate(out_names)
            }
            for c in range(n_cores)
        ]

    def make_chain(n_reps):
        """Jitted callable running the kernel n_reps times back-to-back on
        device (each rep's outputs become the next rep's output buffers),
        for overhead-free timing via slope."""
        def _bodyN(*args):
            ins = list(args[:n_params])
            outs = list(args[n_params:])
            for _ in range(n_reps):
                outs = list(_body(*ins, *outs))
            return tuple(outs)

        return jax.jit(
            shard_map(_bodyN, mesh=mesh, in_specs=specs,
                      out_specs=(PartitionSpec("core"),) * len(out_names),
                      check_rep=False),
            keep_unused=True,
        )

    run.sharded = sharded
    run.in_names = in_names
    run.out_names = out_names
    run.zero_outs = zero_outs
    run.n_params = n_params
    run.make_chain = make_chain
    return run


def _get_runner(reps=1, phase="all"):
    key = ("runner", reps, phase)
    if key not in _CACHE:
        _CACHE[key] = _make_runner(_get_nc(reps, phase), N_CORES)
    return _CACHE[key]


def kernel(**inputs):
    feature_in = np.ascontiguousarray(np.asarray(inputs["feature_in"], np.float32))
    consts = _host_constants(
        np.asarray(inputs["w1"]), np.asarray(inputs["b1"]),
        np.asarray(inputs["w2"]), np.asarray(inputs["b2"]),
        np.asarray(inputs["wf"]), np.asarray(inputs["bf"]),
        np.asarray(inputs["wf2"]), np.asarray(inputs["bf2"]),
    )
    in_maps = []
    for core in range(N_CORES):
        m = {"feat": feature_in[BL * core : BL * core + BL]}
        m.update(consts)
        in_maps.append(m)

    run = _get_runner()
    res = run(in_maps)
    out = np.concatenate([res[c]["outp"] for c in range(N_CORES)], axis=0)
    return out.reshape(B, C, T, F).astype(np.float32)



# revision 17
# speedup vs baseline: 2.1923x; 2.1923x over previous
"""Trainium2 Bass kernel for the DTFA (dual-attention SE + threshold
decomposition) module — bf16 resident-input version.

Math (per batch b):
  zt = SE(mean_T(x))            # [C, F]
  zf = SE(mean_F(x))            # [C, T]
  out1[t,f] = sum_c wf[c]*zf[c,t]*zt[c,f] + bf          (rank-C matmul)
  dcomp[k]  = where(out1 > thr_k, out1, 0), k=1..23
  out[c]    = (sum_k wf2[c,k]*dcomp[k] + bf2[c]) * x[c]

Sharding: pure data-parallel, 2 batches per core on 8 cores.

The tolerance (2e-2 rel) admits a bf16 data path: the input is converted
to bf16 on the host and streamed ONCE into a resident SBUF tile
X[(b,c)=128, 65536 px] (128 KiB/partition).  T-sums are PE
transpose-accumulations into PSUM, F-sums are bf16 pair-add trees split
across DVE and Pool.  SE gates run as block-diagonal matmuls covering
both batches at once.  Pass 2 broadcasts out1 into (2 blk x 2 b x 24 k)
rows per 512-px block (bf16 matmul), thresholds with one
scalar_tensor_tensor, applies the block-diagonal [bf2|wf2] matmul, and
multiplies into X in place (DVE/Pool/ACT split).  The bf16 result is
DMA'd out and upconverted on the host.
"""

import numpy as np
import ml_dtypes

B, C, OC, T, F = 16, 64, 16, 256, 256
N_THR = 23
N_CORES = 8
BL = B // N_CORES  # local batches per core = 2
PIX = T * F        # 65536 per batch
NCH = 8            # input stream chunks
CHW = PIX // NCH   # 8192 cols per chunk

_CACHE = {}


def _host_constants(w1, b1, w2, b2, wf, bf, wf2, bf2):
    f32 = np.float32
    bf16d = ml_dtypes.bfloat16
    bf_s = float(np.asarray(bf).reshape(-1)[0])

    # Broadcast matmul weights: xB[m, n] = sum_r bcastW[r, m] * xflat[r, n]
    # xflat rows: 0 = A even-block px, 1 = B even, 2 = A odd, 3 = B odd,
    #             4 = ones.
    # xB rows m: 0-47 -> even block (g=0), 64-111 -> odd block (g=1);
    # within a 48-group: r = b*24 + k.  k=0 is the bias/ones row.
    bcastW = np.zeros((5, 112), f32)
    for m in range(112):
        if 48 <= m < 64:
            continue
        g, r = (0, m) if m < 48 else (1, m - 64)
        b_loc, k = divmod(r, 24)
        if k == 0:
            bcastW[4, m] = 1.0
        else:
            bcastW[2 * g + b_loc, m] = 1.0
            bcastW[4, m] = bf_s

    # Threshold column for the fused (x > thr) * x op.
    thrcol = np.zeros((112, 1), f32)
    for m in range(112):
        if 48 <= m < 64:
            thrcol[m, 0] = 1e30
            continue
        r = m if m < 48 else m - 64
        k = r % 24
        thrcol[m, 0] = -1e30 if k == 0 else f32(k * (k + 1) / 600.0)

    # Block-diagonal [bf2 | wf2] weights for the decomposition matmul.
    wbd = np.zeros((112, 128), f32)
    for base in (0, 64):
        for b_loc in range(2):
            for k in range(24):
                row = base + 24 * b_loc + k
                cols = slice(64 * b_loc, 64 * b_loc + 64)
                wbd[row, cols] = bf2.astype(f32) if k == 0 else wf2[:, k - 1].astype(f32)

    # Block-diagonal SE weights (both batches in one matmul).  The 1/256
    # mean scale is folded into W1.
    W1bd = np.zeros((128, 32), f32)
    W2bd = np.zeros((32, 128), f32)
    for b_loc in range(2):
        W1bd[64 * b_loc : 64 * b_loc + 64, 16 * b_loc : 16 * b_loc + 16] = (
            w1.T.astype(f32) / 256.0
        )
        W2bd[16 * b_loc : 16 * b_loc + 16, 64 * b_loc : 64 * b_loc + 64] = (
            w2.T.astype(f32)
        )

    packB = np.zeros((128, 528), bf16d)
    packB[0:128, 0:32] = W1bd.astype(bf16d)
    packB[0:32, 32:160] = W2bd.astype(bf16d)
    packB[0:112, 160:288] = wbd.astype(bf16d)
    packB[0:5, 288:400] = bcastW.astype(bf16d)
    packB[:, 400:528] = np.eye(128, dtype=bf16d)

    packA = np.zeros((128, 8), f32)
    packA[0:32, 0:1] = np.concatenate([b1, b1]).astype(f32).reshape(32, 1)
    packA[0:128, 1:2] = np.concatenate([b2, b2]).astype(f32).reshape(128, 1)
    packA[0:128, 2:3] = np.concatenate(
        [wf.reshape(-1), wf.reshape(-1)]
    ).astype(f32).reshape(128, 1)
    packA[0:112, 3:4] = thrcol
    return {"packA": packA, "packB": packB}


CONST_SHAPES = {"packA": ((128, 8), "f32"), "packB": ((128, 528), "bf16")}


def _make_in_map(feature_in_f32, core, consts):
    """Per-core input map. feature_in_f32: full [B, C, T, F] float32."""
    bf16d = ml_dtypes.bfloat16
    sl = feature_in_f32[BL * core : BL * core + BL]
    m = {"feat": np.ascontiguousarray(sl.reshape(BL * C, PIX).astype(bf16d))}
    m.update(consts)
    return m


def _build_nc(reps=1, unroll=False):
    from contextlib import ExitStack, nullcontext

    import concourse.bass as bass
    import concourse.bacc as bacc
    import concourse.tile as tile
    from concourse import mybir

    f32 = mybir.dt.float32
    bf16 = mybir.dt.bfloat16
    Alu = mybir.AluOpType
    Act = mybir.ActivationFunctionType

    nc = bacc.Bacc("TRN2", target_bir_lowering=False, debug=False)
    feat = nc.dram_tensor("feat", [BL * C, PIX], bf16, kind="ExternalInput")
    outp = nc.dram_tensor("outp", [BL * C, PIX], bf16, kind="ExternalOutput")
    ctA = nc.dram_tensor("packA", [128, 8], f32, kind="ExternalInput")
    ctB = nc.dram_tensor("packB", [128, 528], bf16, kind="ExternalInput")

    with tile.TileContext(nc) as tc, ExitStack() as ctx:
        cpool = ctx.enter_context(tc.tile_pool(name="consts", bufs=1))
        cA = cpool.tile([128, 8], f32, tag="packA", name="c_packA")
        nc.gpsimd.dma_start(out=cA[:], in_=ctA[:])
        cB = cpool.tile([128, 528], bf16, tag="packB", name="c_packB")
        nc.gpsimd.dma_start(out=cB[:], in_=ctB[:])
        sb = {
            "b1c": cA[0:32, 0:1], "b2c": cA[0:128, 1:2],
            "wfcol": cA[0:128, 2:3], "thrcol": cA[0:112, 3:4],
            "W1bd": cB[0:128, 0:32], "W2bd": cB[0:32, 32:160],
            "wbd": cB[0:112, 160:288], "bcastW": cB[0:5, 288:400],
            "ident": cB[:, 400:528],
        }

        prepool = ctx.enter_context(tc.tile_pool(name="prep", bufs=1))
        xflat = prepool.tile([5, 16384], bf16, tag="xflat", name="xflat")
        nc.gpsimd.memset(xflat[0:5, :], 1.0)

        loop_cm = (
            tc.For_i(0, reps, 1) if reps > 1 and not unroll else nullcontext()
        )
        ctx.enter_context(loop_cm)
        n_unroll = reps if unroll else 1
        persist = ctx.enter_context(tc.tile_pool(name="persist", bufs=1))

        # Resident input (and, after pass 2, output) tile.
        X = persist.tile([128, PIX], bf16, tag="X", name="X")
        # Gates / sums / out1 staging.
        gates = persist.tile([128, 1792], bf16, tag="gates", name="gates")
        ztsum = gates[:, 0:256]      # [(b,c), f] T-sums
        zfsum = gates[:, 256:512]    # [(b,c), t] F-sums
        zt_g = gates[:, 512:768]
        zf_g = gates[:, 768:1024]
        wfzf = gates[:, 1024:1280]
        ttT = gates[:, 1280:1536]    # transposed T-sums [f, (b,c)]
        h1s = gates[0:32, 1536:1792]
        x_sb = persist.tile([128, 1024], bf16, tag="x_sb", name="x_sb")

        # F-sum scratch, one set per engine.
        fpool = ctx.enter_context(tc.tile_pool(name="fscr", bufs=1))
        fs_s1, fs_s2, fs_s3 = {}, {}, {}
        for e in "vp":
            fs_s1[e] = fpool.tile([128, 4096], bf16, tag=f"s1{e}", name=f"s1{e}")
            fs_s2[e] = fpool.tile([128, 2048], bf16, tag=f"s2{e}", name=f"s2{e}")
            fs_s3[e] = fpool.tile([128, 1024], bf16, tag=f"s3{e}", name=f"s3{e}")

        dcpool = ctx.enter_context(tc.tile_pool(name="dcomp", bufs=2))
        gspool = ctx.enter_context(tc.tile_pool(name="gs", bufs=2))

        FS_ENG = ["v"] * NCH  # per chunk (Pool has no 2x bf16 mode)
        # engine schedules (i % 8): dc threshold op and final multiply
        DC_ENG = ["v"] * 8
        FIN_ENG = ["v", "q", "a", "q", "v", "q", "v", "q"]
        pitch = x_sb.ap[0][0]

        def fill_xflat(H):
            pitch_xf = xflat.ap[0][0]
            for par, (b_loc, off) in enumerate([(0, 0), (1, 0), (0, 2), (1, 2)]):
                for sub in range(2):
                    s0 = x_sb[off + sub : off + sub + 1,
                              256 * (2 * b_loc + H) : 256 * (2 * b_loc + H) + 256]
                    src_ap = bass.AP(
                        tensor=s0.tensor, offset=s0.offset,
                        ap=[[4 * pitch, 32], [1, 256]],
                    )
                    d0 = xflat[par : par + 1, :]
                    dst_ap = bass.AP(
                        tensor=d0.tensor, offset=d0.offset + 256 * sub,
                        ap=[[pitch_xf, 1], [512, 32], [1, 256]],
                    )
                    nc.sync.dma_start(out=dst_ap, in_=src_ap)

        def emit_iter(rep):
            # ---------------- Pass 1: stream in, T/F sums ----------------
            pst_cm = tc.tile_pool(name=f"ps_t{rep}", bufs=1, space="PSUM")
            pst = pst_cm.__enter__()
            tps = [pst.tile([128, 128], bf16, tag=f"tps{h}", name=f"tps{h}")
                   for h in range(2)]
            for q in range(NCH):
                ch = X[:, CHW * q : CHW * q + CHW]
                eng = nc.sync if q % 2 == 0 else nc.scalar
                eng.dma_start(out=ch, in_=feat[:, CHW * q : CHW * q + CHW])

                # T-sums: accumulate transposed 128-px slices into PSUM.
                for j in range(CHW // 256):  # 32 t-rows per chunk
                    t = 32 * q + j
                    for h in range(2):
                        nc.tensor.matmul(
                            tps[h][:],
                            X[:, 256 * t + 128 * h : 256 * t + 128 * h + 128],
                            sb["ident"],
                            is_transpose=True,
                            start=(t == 0), stop=(t == T - 1),
                        )

                # F-sums: bf16 pair-add tree + final reduce.
                e = FS_ENG[q]
                veng = nc.vector if e == "v" else nc.gpsimd
                c3 = ch.rearrange("p (a b) -> p a b", a=32)
                s1 = fs_s1[e][:].rearrange("p (a b) -> p a b", a=32)
                s2 = fs_s2[e][:].rearrange("p (a b) -> p a b", a=32)
                s3 = fs_s3[e][:].rearrange("p (a b) -> p a b", a=32)
                with nc.allow_low_precision(reason="bf16 mean tree"):
                    veng.tensor_tensor(out=s1, in0=c3[:, :, 0:128],
                                       in1=c3[:, :, 128:256], op=Alu.add)
                    veng.tensor_tensor(out=s2, in0=s1[:, :, 0:64],
                                       in1=s1[:, :, 64:128], op=Alu.add)
                    veng.tensor_tensor(out=s3, in0=s2[:, :, 0:32],
                                       in1=s2[:, :, 32:64], op=Alu.add)
                    nc.vector.tensor_reduce(
                        out=zfsum[:, 32 * q : 32 * q + 32], in_=s3,
                        axis=mybir.AxisListType.X, op=Alu.add,
                    )

            # ---------------- T-sum finalize + SE + out1 ----------------
            with tc.tile_pool(name=f"ps_se{rep}", bufs=1, space="PSUM") as ppse:
                for h in range(2):
                    nc.scalar.copy(ttT[:, 128 * h : 128 * h + 128], tps[h][:])
                for h in range(2):
                    tb = ppse.tile([128, 128], bf16, tag="tb", name="tb")
                    nc.tensor.matmul(tb[:], ttT[:, 128 * h : 128 * h + 128],
                                     sb["ident"], is_transpose=True)
                    nc.scalar.copy(ztsum[:, 128 * h : 128 * h + 128], tb[:])

                for zin, zout in ((ztsum, zt_g), (zfsum, zf_g)):
                    h1p = ppse.tile([32, 256], f32, tag="h1p", name="h1p")
                    nc.tensor.matmul(h1p[:], sb["W1bd"], zin)
                    nc.scalar.activation(h1s, h1p[:], Act.Relu,
                                         bias=sb["b1c"], scale=1.0)
                    h2p = ppse.tile([128, 256], f32, tag="h2p", name="h2p")
                    nc.tensor.matmul(h2p[:], sb["W2bd"], h1s)
                    nc.scalar.activation(zout, h2p[:], Act.Sigmoid,
                                         bias=sb["b2c"], scale=1.0)
                nc.vector.tensor_scalar_mul(wfzf, zf_g, sb["wfcol"])
                for b in range(BL):
                    for m in range(2):
                        o1 = ppse.tile([128, 256], f32, tag="o1", name="o1")
                        nc.tensor.matmul(
                            o1[:],
                            wfzf[64 * b : 64 * b + 64, 128 * m : 128 * m + 128],
                            zt_g[64 * b : 64 * b + 64, :],
                        )
                        nc.scalar.copy(
                            x_sb[:, 256 * (2 * b + m) : 256 * (2 * b + m) + 256],
                            o1[:])

            pst_cm.__exit__(None, None, None)

            # ---------------- Pass 2 ----------------
            p2cm_xb = tc.tile_pool(name=f"ps_xb{rep}", bufs=2, space="PSUM")
            ppxb = p2cm_xb.__enter__()
            p2cm_g = tc.tile_pool(name=f"ps_g{rep}", bufs=2, space="PSUM")
            ppg = p2cm_g.__enter__()
            for i in range(64):
                H, i_loc = divmod(i, 32)
                if i_loc == 0:
                    fill_xflat(H)
                xB = ppxb.tile([112, 512], f32, tag="xB", name="xB")
                nc.tensor.matmul(
                    xB[:], sb["bcastW"],
                    xflat[0:5, 512 * i_loc : 512 * i_loc + 512]
                )
                xBs = dcpool.tile([112, 512], bf16, tag="xBs", name="xBs")
                nc.scalar.copy(xBs[:], xB[:])
                dc = dcpool.tile([112, 512], bf16, tag="dc", name="dc")
                deng = nc.vector if DC_ENG[i % 8] == "v" else nc.gpsimd
                with nc.allow_low_precision(reason="bf16 dcomp"):
                    deng.scalar_tensor_tensor(
                        out=dc[:], in0=xBs[:], scalar=sb["thrcol"], in1=xBs[:],
                        op0=Alu.is_gt, op1=Alu.mult,
                    )
                gp = ppg.tile([128, 1024], f32, tag="gp", name="gp")
                for g in (0, 1):
                    nc.tensor.matmul(
                        gp[:, 512 * g : 512 * g + 512],
                        sb["wbd"][64 * g : 64 * g + 48, :],
                        dc[64 * g : 64 * g + 48, :],
                    )
                xi = X[:, 1024 * i : 1024 * i + 1024]
                fe = FIN_ENG[i % 8]
                with nc.allow_low_precision(reason="bf16 output"):
                    if fe == "v":
                        nc.vector.tensor_tensor(out=xi, in0=gp[:], in1=xi,
                                                op=Alu.mult)
                    else:
                        gs = gspool.tile([128, 1024], bf16, tag="gs", name="gs")
                        nc.scalar.copy(gs[:], gp[:])
                        if fe == "a":
                            nc.vector.tensor_tensor(out=xi, in0=gs[:], in1=xi,
                                                    op=Alu.mult)
                        else:
                            nc.gpsimd.tensor_tensor(
                                out=xi, in0=gs[:], in1=xi, op=Alu.mult,
                            )
                if i % 8 == 7:
                    j = i // 8
                    eng = nc.sync if j % 2 == 0 else nc.scalar
                    eng.dma_start(out=outp[:, CHW * j : CHW * j + CHW],
                                  in_=X[:, CHW * j : CHW * j + CHW])
            p2cm_g.__exit__(None, None, None)
            p2cm_xb.__exit__(None, None, None)

        for rep in range(n_unroll):
            emit_iter(rep)

    nc.finalize()
    return nc


def _get_nc(reps=1, phase="all", unroll=False):
    key = ("nc", reps, unroll)
    if key not in _CACHE:
        _CACHE[key] = _build_nc(reps, unroll)
    return _CACHE[key]


def _make_runner(nc, n_cores):
    """Cached jitted shard_map executor for `nc` (mirrors
    bass2jax.run_bass_via_pjrt but reusable across calls)."""
    import jax
    from jax.sharding import Mesh, PartitionSpec
    from jax.experimental.shard_map import shard_map
    from concourse import bass2jax, mybir

    bass2jax.install_neuronx_cc_hook()

    partition_name = (
        nc.partition_id_tensor.name if nc.partition_id_tensor else None
    )
    in_names, out_names, out_avals, zero_outs = [], [], [], []
    for alloc in nc.m.functions[0].allocations:
        if not isinstance(alloc, mybir.MemoryLocationSet):
            continue
        name = alloc.memorylocations[0].name
        if alloc.kind == "ExternalInput":
            if name != partition_name:
                in_names.append(name)
        elif alloc.kind == "ExternalOutput":
            out_names.append(name)
            shape = tuple(alloc.tensor_shape)
            dtype = mybir.dt.np(alloc.dtype)
            out_avals.append(jax.core.ShapedArray(shape, dtype))
            zero_outs.append(np.zeros(shape, dtype))
    n_params = len(in_names)
    all_in_names = in_names + out_names
    if partition_name is not None:
        all_in_names = all_in_names + [partition_name]
    donate = tuple(range(n_params, n_params + len(out_names)))

    def _body(*args):
        operands = list(args)
        if partition_name is not None:
            operands.append(bass2jax.partition_id_tensor())
        outs = bass2jax._bass_exec_p.bind(
            *operands,
            out_avals=tuple(out_avals),
            in_names=tuple(all_in_names),
            out_names=tuple(out_names),
            lowering_input_output_aliases=(),
            sim_require_finite=True,
            sim_require_nnan=True,
            nc=nc,
        )
        return tuple(outs)

    devices = jax.devices()[:n_cores]
    mesh = Mesh(np.asarray(devices), ("core",))
    specs = (PartitionSpec("core"),) * (n_params + len(out_names))
    sharded = jax.jit(
        shard_map(_body, mesh=mesh, in_specs=specs,
                  out_specs=(PartitionSpec("core"),) * len(out_names),
                  check_rep=False),
        donate_argnums=donate, keep_unused=True,
    )

    def run(in_maps):
        per_core = [[np.asarray(m[name]) for name in in_names] for m in in_maps]
        concat_in = [
            np.concatenate([per_core[c][i] for c in range(n_cores)], axis=0)
            for i in range(n_params)
        ]
        out_arrs = sharded(*concat_in, *[
            np.zeros((n_cores * z.shape[0], *z.shape[1:]), z.dtype)
            for z in zero_outs
        ])
        return [
            {
                name: np.asarray(out_arrs[i]).reshape(n_cores, *out_avals[i].shape)[c]
                for i, name in enumerate(out_names)
            }
            for c in range(n_cores)
        ]

    run.sharded = sharded
    run.in_names = in_names
    run.out_names = out_names
    run.zero_outs = zero_outs
    run.n_params = n_params
    return run


def _get_runner(reps=1, phase="all"):
    key = ("runner", reps)
    if key not in _CACHE:
        _CACHE[key] = _make_runner(_get_nc(reps), N_CORES)
    return _CACHE[key]


def kernel(**inputs):
    feature_in = np.ascontiguousarray(np.asarray(inputs["feature_in"], np.float32))
    consts = _host_constants(
        np.asarray(inputs["w1"]), np.asarray(inputs["b1"]),
        np.asarray(inputs["w2"]), np.asarray(inputs["b2"]),
        np.asarray(inputs["wf"]), np.asarray(inputs["bf"]),
        np.asarray(inputs["wf2"]), np.asarray(inputs["bf2"]),
    )
    in_maps = [_make_in_map(feature_in, core, consts) for core in range(N_CORES)]

    run = _get_runner()
    res = run(in_maps)
    out = np.concatenate(
        [np.asarray(res[c]["outp"]).astype(np.float32) for c in range(N_CORES)],
        axis=0,
    )
    return out.reshape(B, C, T, F)
